# revision 2
# baseline (speedup 1.0000x reference)
"""ChebConv (K=3) kernel for Trainium2, data-parallel over batch across 8 NeuronCores.

Math (per batch b):
    d    = adj.sum(axis=1)  (row sums), dinv = (d+eps)^-0.5, dsq = (d+eps)^0.5
    M    = Dinv A Dinv  (L = I - M)
    Tx0 = x, Tx1 = L x, Tx2 = 2 L Tx1 - Tx0
    out  = relu(sum_k Txk @ W[k] + sum_k b[k])

Kernel-side reformulation with u_k := Dsq Tx_k and UNSCALED at2[j, i] = A[i, j]:
every PE pass contracts a Dinv^2-scaled natural operand against plain A^T, so
the Chebyshev recurrence is elementwise in the transposed domain:
    u0  = Dsq x,  yp0 = Dinv x  (= Dinv^2 u0)
    z1T = pass(yp0) : z1T[f,i] = sum_j yp0[j,f] at2[j,i]   ( = (Dsq M x)^T )
    u1T = u0T - z1T
    y1s[c] = dinv2[c] * u1n[c]
    z2T = pass(y1s)                                  ( = (Dsq M Tx1)^T )
    u2T = 2 u1T - u0T - 2 z2T
    out = relu(Dinv @ (sum_k u_k @ W[k]) + bsum)     (lhsT = u_kT blocks)

V2 streaming: adj is loaded via SWDGE cast-DMAs (gpsimd ring) that convert
f32 -> bf16 inline in the SDMA datapath at full HBM rate (~348 GB/s measured),
eliminating the DVE/ACT cast passes entirely.  Row sums are computed from the
bf16 strips (DVE tensor_reduce / ACT identity+accum, alternating).  The PE is
pre-warmed with dummy matmuls so the HAM clock gate sits at K=8/8 before the
first transpose.  The tail runs pass 2 contraction-major for the first 12
steps (so it can start while y1s is still being produced), then output-chunk
-major for the last 4 so each z2 chunk's epilogue (u2T, output matmuls, relu,
store) overlaps the remaining contraction.
"""

import numpy as np

B, N, F, K = 8, 2048, 128, 3
P = 128
NT = N // P  # 16
CH = 8       # adj DMA chunks
SPC = NT // CH  # strips per chunk = 2
EPS = 1e-6
NCORES = 8
WARMUP_MM = 64

_cache = {}


def _build_nc():
    from contextlib import ExitStack

    import concourse.bacc as bacc
    import concourse.tile as tile
    from concourse import mybir

    f32 = mybir.dt.float32
    bf16 = mybir.dt.bfloat16
    AF = mybir.ActivationFunctionType
    OP = mybir.AluOpType
    AX = mybir.AxisListType

    nc = bacc.Bacc("TRN2", target_bir_lowering=False, debug=False, num_devices=NCORES)
    adj = nc.dram_tensor("adj", [N, N], f32, kind="ExternalInput").ap()
    x = nc.dram_tensor("x", [N, F], f32, kind="ExternalInput").ap()
    wb_d = nc.dram_tensor("wb", [P, K, F], bf16, kind="ExternalInput").ap()
    bsum_d = nc.dram_tensor("bsum", [P, F], f32, kind="ExternalInput").ap()
    identb_d = nc.dram_tensor("identb", [P, P], bf16, kind="ExternalInput").ap()
    out = nc.dram_tensor("out", [N, F], f32, kind="ExternalOutput").ap()
    out_t = out.rearrange("(t p) f -> p t f", p=P)
    adj_r = adj.rearrange("(c t p) j -> c p t j", p=P, t=SPC)
    x_r = x.rearrange("(t p) f -> p t f", p=P)

    with ExitStack() as ctx:
        tc = ctx.enter_context(tile.TileContext(nc))
        consts = ctx.enter_context(tc.tile_pool(name="consts", bufs=1))
        abp = ctx.enter_context(tc.tile_pool(name="abp", bufs=3))
        scr = ctx.enter_context(tc.tile_pool(name="scr", bufs=2))
        big = ctx.enter_context(tc.tile_pool(name="big", bufs=1))
        small = ctx.enter_context(tc.tile_pool(name="small", bufs=4))
        ps_acc = ctx.enter_context(tc.tile_pool(name="ps_acc", bufs=1, space="PSUM"))
        ps_t = ctx.enter_context(tc.tile_pool(name="ps_t", bufs=4, space="PSUM"))

        # ---- constants (scalar ring) + x (sync ring, one 1MB DMA) -------
        ident_bf = consts.tile([P, P], bf16)
        nc.scalar.dma_start(out=ident_bf, in_=identb_d)
        w_bf = consts.tile([P, K, F], bf16)
        nc.scalar.dma_start(out=w_bf, in_=wb_d)
        bsum = consts.tile([P, F], f32)
        nc.scalar.dma_start(out=bsum, in_=bsum_d)
        x_t = consts.tile([P, NT, F], f32)
        nc.sync.dma_start(out=x_t, in_=x_r)
        eps_sb = consts.tile([P, 1], f32)
        nc.vector.memset(eps_sb, EPS)

        dinv = consts.tile([P, NT], f32)
        dinv2 = consts.tile([P, NT], f32)

        u0 = big.tile([P, NT, F], bf16)    # Dsq x, natural (u0T source)
        yp0 = big.tile([P, NT, F], bf16)   # Dinv x, natural (pass-1 lhsT)
        y1s = big.tile([P, NT, F], bf16)   # dinv2 * u1, natural (pass-2 lhsT)
        at2 = big.tile([P, NT, N], bf16)   # [j_in_tile, c(j tile), i]: A[i,j]
        uT0 = big.tile([P, N], bf16)       # transposed u0: [f, i]
        uT1 = big.tile([P, N], bf16)
        uT2 = big.tile([P, N], bf16)
        ttT = big.tile([P, N], bf16)       # 2*u1T - u0T

        z1 = ps_acc.tile([P, N], f32, tag="acc")

        # ---- PE warm-up: dummy matmuls ride the HAM SHORT window so the
        #      first real transposes run at 2.4 GHz ------------------------
        wu = ps_t.tile([P, 4, P], f32, tag="t")
        for i in range(WARMUP_MM):
            nc.tensor.matmul(wu[:, i % 4, :], lhsT=ident_bf, rhs=ident_bf,
                             start=True, stop=True, skip_group_check=True)

        def emit_u0T_and_pass1(r):
            # u0T strip r (transpose u0 via PE)
            pt0 = ps_t.tile([P, 4, P], f32, tag="t")
            nc.tensor.matmul(pt0[:, 0, :], lhsT=u0[:, r, :], rhs=ident_bf,
                             start=True, stop=True)
            if r % 2 == 0:
                nc.vector.tensor_copy(out=uT0[:, r * P:(r + 1) * P],
                                      in_=pt0[:, 0, :])
            else:
                nc.scalar.copy(out=uT0[:, r * P:(r + 1) * P], in_=pt0[:, 0, :])
            # triangular pass-1 terms that became ready with strip r:
            # (a) older strips s < r with new weight block c = r (bank chunks)
            for sg in range((r + 3) // 4):
                lo = 4 * sg
                hi = min(lo + 4, r)  # strips [lo, hi)
                nc.tensor.matmul(z1[:, lo * P:hi * P], lhsT=yp0[:, r, :],
                                 rhs=at2[:, r, lo * P:hi * P],
                                 start=False, stop=(r == NT - 1),
                                 skip_group_check=True)
            # (b) strip r, weight blocks c <= r
            for c in range(r + 1):
                nc.tensor.matmul(z1[:, r * P:(r + 1) * P], lhsT=yp0[:, c, :],
                                 rhs=at2[:, c, r * P:(r + 1) * P],
                                 start=(r % 4 == 0 and c == 0),
                                 stop=(c == NT - 1), skip_group_check=True)

        # ---- streaming phase: SWDGE cast-DMA chunks of 2 strips ---------
        for cc in range(CH):
            a_t = abp.tile([P, SPC, N], bf16, tag="a")
            nc.gpsimd.dma_start(out=a_t, in_=adj_r[cc])
            for t in range(SPC):
                r = SPC * cc + t
                # row sum of bf16 strip (DVE / ACT alternating)
                d_r = small.tile([P, 1], f32, tag="d")
                if t == 0:
                    nc.vector.tensor_reduce(out=d_r, in_=a_t[:, t, :],
                                            axis=AX.X, op=OP.add)
                else:
                    s_t = scr.tile([P, N], bf16, tag="s")
                    nc.scalar.activation(out=s_t, in_=a_t[:, t, :],
                                         func=AF.Identity, accum_out=d_r)

                # transpose A strip r on the PE (4 PSUM groups of 4 blocks)
                for g in range(4):
                    pt = ps_t.tile([P, 4, P], f32, tag="t")
                    for q in range(4):
                        c = 4 * g + q
                        nc.tensor.matmul(pt[:, q, :],
                                         lhsT=a_t[:, t, c * P:(c + 1) * P],
                                         rhs=ident_bf, start=True, stop=True)
                    if g % 2 == 0:
                        nc.vector.tensor_copy(
                            out=at2[:, 4 * g:4 * g + 4, r * P:(r + 1) * P],
                            in_=pt)
                    else:
                        nc.scalar.copy(
                            out=at2[:, 4 * g:4 * g + 4, r * P:(r + 1) * P],
                            in_=pt)

                # scalar chain for strip r
                dsq_r = small.tile([P, 1], f32, tag="dsq")
                nc.scalar.activation(out=dsq_r, in_=d_r, func=AF.Sqrt,
                                     bias=eps_sb)
                nc.vector.reciprocal(out=dinv[:, r:r + 1], in_=dsq_r)
                nc.scalar.mul(out=u0[:, r, :], in_=x_t[:, r, :], mul=dsq_r)
                nc.scalar.mul(out=yp0[:, r, :], in_=x_t[:, r, :],
                              mul=dinv[:, r:r + 1])

                # PE work depending on strip r-1's scalar chain, emitted here
                # so strip r's transposes sit ahead of it in the PE queue.
                if r >= 1:
                    emit_u0T_and_pass1(r - 1)
        emit_u0T_and_pass1(NT - 1)

        # dinv2 = dinv*dinv, all 16 strips in one bulk op
        nc.vector.scalar_tensor_tensor(out=dinv2, in0=dinv, scalar=1.0,
                                       in1=dinv, op0=OP.mult, op1=OP.mult)

        # ---- recurrence: u1T = u0T - z1T (pure DVE) ---------------------
        for ch in range(4):
            s = slice(ch * 512, (ch + 1) * 512)
            nc.vector.scalar_tensor_tensor(
                out=uT1[:, s], in0=z1[:, s], scalar=-1.0, in1=uT0[:, s],
                op0=OP.mult, op1=OP.add)

        # ---- pass 2 prologue: y1s[c] = dinv2[c]*u1n[c] ------------------
        z2 = ps_acc.tile([P, N], f32, tag="acc")
        ptcs = {}

        def emit_ptc(c):
            ptc = ps_t.tile([P, 4, P], f32, tag="t")
            nc.tensor.matmul(ptc[:, 0, :], lhsT=uT1[:, c * P:(c + 1) * P],
                             rhs=ident_bf, start=True, stop=True)
            ptcs[c] = ptc

        def emit_y1s(c):
            if c % 2 == 0:
                nc.vector.tensor_scalar(out=y1s[:, c, :], in0=ptcs[c][:, 0, :],
                                        scalar1=dinv2[:, c:c + 1],
                                        scalar2=None, op0=OP.mult)
            else:
                nc.scalar.mul(out=y1s[:, c, :], in_=ptcs[c][:, 0, :],
                              mul=dinv2[:, c:c + 1])
            del ptcs[c]

        emit_ptc(0)
        emit_y1s(0)
        emit_ptc(1)
        emit_y1s(1)

        # ttT = 2*u1T - u0T (DVE, overlaps pass-2 prologue)
        for ch in range(4):
            s = slice(ch * 512, (ch + 1) * 512)
            nc.vector.scalar_tensor_tensor(
                out=ttT[:, s], in0=uT1[:, s], scalar=2.0, in1=uT0[:, s],
                op0=OP.mult, op1=OP.subtract)

        # ---- pass 2, stage 1: contraction-major over c = 0..11 so the PE
        #      can start while y1s c-blocks are still being produced ------
        CSPLIT = NT - 4
        for c in range(CSPLIT):
            if c + 2 < NT:
                emit_ptc(c + 2)
                emit_y1s(c + 2)
            for nch in range(4):
                nc.tensor.matmul(z2[:, nch * 512:(nch + 1) * 512],
                                 lhsT=y1s[:, c, :],
                                 rhs=at2[:, c, nch * 512:(nch + 1) * 512],
                                 start=(c == 0), stop=False,
                                 skip_group_check=True)

        # ---- pass 2, stage 2: chunk-major over the last 4 contraction
        #      steps; each finished z2 chunk immediately runs its epilogue:
        #      u2T chunk, output matmuls, bias+scale, relu, store ---------
        uTs = (uT0, uT1, uT2)
        for g in range(4):
            s = slice(g * 512, (g + 1) * 512)
            for c in range(CSPLIT, NT):
                nc.tensor.matmul(z2[:, s], lhsT=y1s[:, c, :],
                                 rhs=at2[:, c, s],
                                 start=False, stop=(c == NT - 1),
                                 skip_group_check=True)
            nc.vector.scalar_tensor_tensor(
                out=uT2[:, s], in0=z2[:, s], scalar=-2.0, in1=ttT[:, s],
                op0=OP.mult, op1=OP.add)
            og = small.tile([P, 4, F], f32, tag="og")
            for q in range(4):
                rr = 4 * g + q
                oc = ps_t.tile([P, 4, P], f32, tag="t")
                for k3 in range(K):
                    nc.tensor.matmul(oc[:, 0, :],
                                     lhsT=uTs[k3][:, rr * P:(rr + 1) * P],
                                     rhs=w_bf[:, k3, :],
                                     start=(k3 == 0), stop=(k3 == K - 1))
                tmp = small.tile([P, F], f32, tag="tmp")
                nc.vector.scalar_tensor_tensor(
                    out=tmp, in0=oc[:, 0, :], scalar=dinv[:, rr:rr + 1],
                    in1=bsum, op0=OP.mult, op1=OP.add)
                nc.scalar.activation(out=og[:, q, :], in_=tmp, func=AF.Relu)
            nc.sync.dma_start(out=out_t[:, 4 * g:4 * g + 4, :], in_=og)

    nc.compile()
    return nc


def _get_nc():
    if "nc" not in _cache:
        _cache["nc"] = _build_nc()
    return _cache["nc"]


def make_in_maps(x, adj, W, b):
    import ml_dtypes

    bf16 = ml_dtypes.bfloat16
    identb = np.ascontiguousarray(np.eye(P, dtype=np.float32).astype(bf16))
    x = np.ascontiguousarray(np.asarray(x, dtype=np.float32))
    adj = np.ascontiguousarray(np.asarray(adj, dtype=np.float32))
    # W [K, in, out] -> [in, K, out] bf16 (host-side rearrange + cast)
    wb = np.ascontiguousarray(
        np.asarray(W, dtype=np.float32).transpose(1, 0, 2).astype(bf16))
    bf = np.asarray(b, dtype=np.float32)
    bsum = np.ascontiguousarray(
        np.broadcast_to(bf.sum(axis=0), (P, F)).astype(np.float32))
    return [
        {"adj": adj[c], "x": x[c], "wb": wb, "bsum": bsum, "identb": identb}
        for c in range(NCORES)
    ]


def run_raw(x, adj, W, b, **kwargs):
    from concourse import bass_utils

    nc = _get_nc()
    in_maps = make_in_maps(x, adj, W, b)
    res = bass_utils.run_bass_kernel_spmd(nc, in_maps,
                                          core_ids=list(range(NCORES)), **kwargs)
    out = np.stack([res.results[c]["out"] for c in range(NCORES)], axis=0)
    return out.astype(np.float32), res


def kernel(x, adj, W, b):
    out, _ = run_raw(x, adj, W, b)
    return out
